# revision 24
# baseline (speedup 1.0000x reference)
"""Trainium2 Bass kernel for nn_MultiHeadAttention_T_4818953306886.

Reference semantics (B=8, S=2048, D=64, H=8, HD=512):
    q = (x @ Wq + bq).reshape(B*H, S, D)      # raw row-major view!
    k, v likewise
    attn = softmax(q @ k^T / sqrt(D), axis=2)
    ctx = attn @ v                             # [B*H, S, D]
    out = ctx.reshape(B, 1, S, HD) @ Wo + bo   # [B, 1, S, D]
    y = LayerNorm(x[:, None] + out) * gamma + beta

The raw reshape means head h attends over a permuted view of rows
h*256..h*256+255 of the projection output: with j' = c*256 + r,
    Q'_h[c*256+r, d] = Q[h*256+r, c*64+d]
and softmax attention is invariant to the (shared) permutation of the
key/value index, so we work entirely in (c, r) block layout.

Sharding: data-parallel over batch, one batch per core, weights replicated.

Dataflow per core (matmul operands fp16, accumulation fp32):
  xT   (64, 2048)        x^T (host-pretransposed input)
  QTd  (64, c=8, 2048)   Q^T per 64-wide column chunk (1/sqrt(D) folded in)
  KTd  same for K
  Vaug (128, h=8, jc=16, 65)  V' j'-chunks with a ones column appended
  per (head, i'-tile of 512):
    scoresT (j'=128, i'=512) via K-stationary matmuls -> PSUM
    E = exp(scoresT) on ScalarE (PSUM -> SBUF fp16)
    ctxT_aug (65, 512) accumulated PV matmul; row 64 = softmax denominator
    normalize: DVE reciprocal + partition-broadcast + one DVE multiply
  out-projection per s-tile of 128 (accumulated over the 8 c-chunks) + bo
  (rank-1 matmul), + residual; LayerNorm with a single batched
  rstd = exp(-0.5*ln(var+eps)) pass over all 16 s-tiles.

Projection groups (one 512-wide s-tile of Q/K + 4 V s-tiles) are emitted
interleaved with the head loop so the ScalarE exp stream starts early and
never starves; PSUM stays within the 8-bank budget:
  scores 2x(2 banks) + ctx 2x(1) + aux(proj/outproj/bcast) 2x(1) = 8.
"""

import numpy as np

import concourse.bass as bass
import concourse.tile as tile
from concourse import mybir
from concourse.bass_utils import run_bass_kernel_spmd

F32 = mybir.dt.float32
F16 = mybir.dt.float16

S = 2048          # sequence length per batch
DIN = 64          # model dim
H = 8             # heads
HD = 512          # H * DIN
NT = 16           # s-tiles of 128
P = 128
LN_EPS = 1e-5
AF = mybir.ActivationFunctionType

# scores jc-chunks per exp op: 2 -> (128, 1024) fp32 = 2 PSUM banks
SJC = 2
# use GPSIMD stride-0 partition broadcast for the softmax denominator;
# if False, use a rank-1 PE matmul + DVE copy instead.
GPSIMD_BCAST = False

_CACHE = {}

# walrus in this container accepts at most 1 sync-wait command per engine
# instruction and at most 2 per EventSemaphore. Tile packs every dependency
# onto the consuming instruction, so hoist the excess onto standalone
# EventSemaphore instructions inserted just before it (same engine stream).
_MAX_EV_WAITS = 2


def _legalize_sync_waits(nc, keep=1):
    n_fixed = 0
    for function in nc.m.functions:
        for block in function.blocks:
            out = []
            changed = False
            for inst in block.instructions:
                si = getattr(inst, "sync_info", None)
                waits = list(si.on_wait) if (si is not None and si.on_wait) else []
                if len(waits) > keep and not isinstance(
                        inst, mybir.InstEventSemaphore):
                    extra = waits[:-keep] if keep else waits
                    kept = waits[-keep:] if keep else []
                    for ci in range(0, len(extra), _MAX_EV_WAITS):
                        ev = mybir.InstEventSemaphore(
                            name=f"{inst.name}-w{ci}", ins=[], outs=[],
                            sync_info=mybir.SyncInfo(
                                on_wait=extra[ci:ci + _MAX_EV_WAITS],
                                on_update=[]),
                        )
                        ev.engine = inst.engine
                        out.append(ev)
                    inst.sync_info = mybir.SyncInfo(
                        on_wait=kept, on_update=list(si.on_update))
                    changed = True
                    n_fixed += 1
                out.append(inst)
            if changed:
                block.instructions = out
    return n_fixed


def _build():
    nc = bass.Bass()

    x_d = nc.dram_tensor("x", [P, NT, DIN], F32, kind="ExternalInput")
    xt_d = nc.dram_tensor("xt", [DIN, S], F16, kind="ExternalInput")
    wq_d = nc.dram_tensor("wq", [DIN, HD], F16, kind="ExternalInput")
    wk_d = nc.dram_tensor("wk", [DIN, HD], F16, kind="ExternalInput")
    wv_d = nc.dram_tensor("wv", [DIN, HD], F16, kind="ExternalInput")
    wo_d = nc.dram_tensor("wo", [DIN, H, DIN], F16, kind="ExternalInput")
    bq_d = nc.dram_tensor("bq", [DIN, H], F32, kind="ExternalInput")
    bk_d = nc.dram_tensor("bk", [DIN, H], F32, kind="ExternalInput")
    bv_d = nc.dram_tensor("bv", [1, HD], F16, kind="ExternalInput")
    bo_d = nc.dram_tensor("bo", [1, DIN], F16, kind="ExternalInput")
    gam_d = nc.dram_tensor("gamma", [P, DIN], F32, kind="ExternalInput")
    bet_d = nc.dram_tensor("beta", [P, DIN], F32, kind="ExternalInput")
    y_d = nc.dram_tensor("y", [S, DIN], F32, kind="ExternalOutput")

    with tile.TileContext(nc) as tc:
        with (
            tc.tile_pool(name="consts", bufs=1) as consts,
            tc.tile_pool(name="spool", bufs=2, space=bass.MemorySpace.PSUM) as spool,
            tc.tile_pool(name="cpool", bufs=2, space=bass.MemorySpace.PSUM) as cpool,
            tc.tile_pool(name="aux", bufs=2, space=bass.MemorySpace.PSUM) as aux,
            tc.tile_pool(name="epool", bufs=2) as epool,
            tc.tile_pool(name="wpool", bufs=2) as wpool,
            tc.tile_pool(name="lpool", bufs=3) as lpool,
        ):
            ones0 = consts.tile([1, P], F16)
            nc.vector.memset(ones0[:], 1.0)
            ones2 = consts.tile([65, P], F16)
            nc.vector.memset(ones2[64:65, :], 1.0)
            eps_t = consts.tile([P, 1], F32)
            nc.vector.memset(eps_t, LN_EPS)
            # dummy exp: trigger the ACT table load during the prologue
            warm = consts.tile([P, 1], F32)
            nc.scalar.activation(warm[:], eps_t[:], AF.Exp)

            # DMA order matters: xT + K/Q weights + their biases gate the
            # critical path (first projections -> first scores -> first exp).
            xT = consts.tile([DIN, S], F16)
            nc.sync.dma_start(xT[:], xt_d[:])
            wk_sb = consts.tile([DIN, HD], F16)
            nc.sync.dma_start(wk_sb[:], wk_d[:])
            wq_sb = consts.tile([DIN, HD], F16)
            nc.sync.dma_start(wq_sb[:], wq_d[:])
            bq_sb = consts.tile([DIN, H], F32)
            nc.sync.dma_start(bq_sb[:], bq_d[:])
            bk_sb = consts.tile([DIN, H], F32)
            nc.sync.dma_start(bk_sb[:], bk_d[:])
            wv_sb = consts.tile([DIN, HD], F16)
            nc.sync.dma_start(wv_sb[:], wv_d[:])
            bv_sb = consts.tile([1, HD], F16)
            nc.sync.dma_start(bv_sb[:], bv_d[:])
            wo_sb = consts.tile([DIN, H, DIN], F16)
            nc.sync.dma_start(wo_sb[:], wo_d[:])
            bo_sb = consts.tile([1, DIN], F16)
            nc.sync.dma_start(bo_sb[:], bo_d[:])
            gamma_b = consts.tile([P, DIN], F32)
            nc.sync.dma_start(gamma_b[:], gam_d[:])
            beta_b = consts.tile([P, DIN], F32)
            nc.sync.dma_start(beta_b[:], bet_d[:])
            x_res = consts.tile([P, NT, DIN], F32)
            nc.sync.dma_start(x_res[:], x_d[:])

            QTd = consts.tile([DIN, H, S], F16)
            KTd = consts.tile([DIN, H, S], F16)
            Vaug = consts.tile([P, H, NT, 65], F16)
            nc.vector.memset(Vaug[:, :, :, 64:65], 1.0)
            ctxT = consts.tile([DIN, H, S], F16)

            y_all = consts.tile([P, NT, DIN], F32)
            mv_all = consts.tile([P, NT, 2], F32)
            rstd_all = consts.tile([P, NT], F32)

            def proj_group(g):
                """Q^T, K^T chunks for s-columns [g*512,(g+1)*512) and
                V'/Vaug rows for heads 2g, 2g+1 — as a list of small
                closures (~1 matmul + evacuation each) so they can be
                sprinkled between score-groups without starving ScalarE."""
                sl = slice(g * 512, (g + 1) * 512)
                work = []

                def qk(w_sb, b_sb, out_t, c):
                    ps = aux.tile([DIN, 512], F32, tag="aux")
                    nc.tensor.matmul(
                        ps[:], w_sb[:, c * DIN:(c + 1) * DIN], xT[:, sl],
                        start=True, stop=True)
                    nc.vector.tensor_scalar_add(
                        out_t[:, c, sl], ps[:], b_sb[:, c:c + 1])

                def vproj(k):
                    st = 4 * g + k
                    h, half = st // 2, st % 2
                    ps = aux.tile([P, HD], F32, tag="aux")
                    nc.tensor.matmul(
                        ps[:], xT[:, st * P:(st + 1) * P], wv_sb[:],
                        start=True, stop=False)
                    nc.tensor.matmul(
                        ps[:], ones0[:], bv_sb[:], start=False, stop=True)
                    # psum[p, c*64+d] -> Vaug[p, h, 2c+half, d]
                    dst = bass.AP(
                        tensor=Vaug.tensor,
                        offset=Vaug.offset + (h * NT + half) * 65,
                        ap=[[Vaug.ap[0][0], P], [2 * 65, H], [1, DIN]],
                    )
                    nc.vector.tensor_copy(
                        dst, ps[:].rearrange("p (c d) -> p c d", d=DIN))

                # K chunks first (score-group g of every i'-tile needs K
                # chunk c=g), then V (needed by the first PV accumulation),
                # then remaining Q chunks (needed pairwise per i'-tile).
                for c in range(H):
                    work.append(lambda c=c: qk(wk_sb, bk_sb, KTd, c))
                for c in range(2):
                    work.append(lambda c=c: qk(wq_sb, bq_sb, QTd, c))
                for k in range(4):
                    work.append(lambda k=k: vproj(k))
                for c in range(2, H):
                    work.append(lambda c=c: qk(wq_sb, bq_sb, QTd, c))
                return work

            def attention(h, fillers, rate=1):
                fi = 0
                for it in range(4):
                    rhs_q = QTd[:, 2 * it:2 * it + 2, h * 256:(h + 1) * 256]
                    E_t = epool.tile([P, NT, 512], F16, tag="E")
                    for g in range(NT // SJC):
                        ps = spool.tile([P, SJC * 512], F32, tag="sc")
                        for jg in range(SJC):
                            jc = g * SJC + jg
                            off = h * 256 + (jc % 2) * P
                            nc.tensor.matmul(
                                ps[:, jg * 512:(jg + 1) * 512],
                                KTd[:, jc // 2, off:off + P],
                                rhs_q, start=True, stop=True)
                        nc.scalar.activation(
                            E_t[:, g * SJC:(g + 1) * SJC, :], ps[:], AF.Exp)
                        for _ in range(rate):
                            if fi < len(fillers):
                                fillers[fi]()
                                fi += 1
                    pc = cpool.tile([65, 512], F32, tag="ctx")
                    for jc in range(NT):
                        nc.tensor.matmul(
                            pc[:], Vaug[:, h, jc, :], E_t[:, jc, :],
                            start=(jc == 0), stop=(jc == NT - 1))
                    # row 64 of pc is the softmax denominator over j'
                    rs = wpool.tile([65, 512], F32, tag="r32")
                    nc.vector.reciprocal(rs[64:65, :], pc[64:65, :])
                    if GPSIMD_BCAST:
                        bc = wpool.tile([DIN, 512], F32, tag="bcs")
                        row = rs[64:65, :]
                        src = bass.AP(tensor=row.tensor, offset=row.offset,
                                      ap=[[0, DIN]] + [list(a) for a in row.ap[1:]])
                        nc.gpsimd.tensor_copy(bc[:], src)
                    else:
                        rs16 = wpool.tile([65, 512], F16, tag="r16")
                        nc.vector.tensor_copy(rs16[64:65, :], rs[64:65, :])
                        pb = aux.tile([DIN, 512], F32, tag="aux")
                        nc.tensor.matmul(
                            pb[:], ones2[64:65, 0:DIN], rs16[64:65, :],
                            start=True, stop=True)
                        bc = wpool.tile([DIN, 512], F16, tag="bcs")
                        nc.vector.tensor_copy(bc[:], pb[:])
                    nc.vector.tensor_mul(
                        ctxT[:, 2 * it:2 * it + 2, h * 256:(h + 1) * 256],
                        pc[0:DIN, :].rearrange("d (c r) -> d c r", r=256),
                        bc[:].rearrange("d (c r) -> d c r", r=256))

            def outproj(st):
                po = aux.tile([P, DIN], F32, tag="aux")
                for c in range(H):
                    nc.tensor.matmul(
                        po[:], ctxT[:, c, st * P:(st + 1) * P],
                        wo_sb[:, c, :], start=(c == 0), stop=False)
                nc.tensor.matmul(
                    po[:], ones0[:], bo_sb[:], start=False, stop=True)
                nc.vector.tensor_add(y_all[:, st, :], po[:], x_res[:, st, :])
                stats = lpool.tile([P, 6], F32, tag="st")
                nc.vector.bn_stats(stats[:], y_all[:, st, :])
                nc.vector.bn_aggr(mv_all[:, st, :], stats[:])

            lnv = consts.tile([P, NT], F32)

            def finalize(st0, st1):
                """LayerNorm rstd = exp(-0.5*ln(var+eps)) for s-tiles
                [st0, st1), then normalize + affine + store."""
                nc.scalar.activation(
                    lnv[:, st0:st1], mv_all[:, st0:st1, 1], AF.Ln,
                    bias=eps_t[:])
                nc.scalar.activation(
                    rstd_all[:, st0:st1], lnv[:, st0:st1], AF.Exp, scale=-0.5)
                for st in range(st0, st1):
                    yn = lpool.tile([P, DIN], F32, tag="yn")
                    nc.vector.tensor_scalar(
                        yn[:], y_all[:, st, :],
                        scalar1=mv_all[:, st, 0:1],
                        scalar2=rstd_all[:, st:st + 1],
                        op0=mybir.AluOpType.subtract,
                        op1=mybir.AluOpType.mult)
                    nc.vector.tensor_mul(yn[:], yn[:], gamma_b[:])
                    yo = lpool.tile([P, DIN], F32, tag="yo")
                    nc.vector.tensor_add(yo[:], yn[:], beta_b[:])
                    nc.sync.dma_start(y_d[st * P:(st + 1) * P, :], yo[:])

            # projection group 0: K c0 + Q c0,c1 run up front (the minimum
            # head 0's first score-group needs); everything else — remaining
            # projections, the next group, the previous head's output
            # projections, and the first LayerNorm finalize — is sprinkled
            # through the head loop as fillers so the PE never takes a
            # detour longer than ScalarE's score backlog.
            g0 = proj_group(0)
            g0[0]()   # K c0
            g0[8]()   # Q c0
            g0[9]()   # Q c1
            pre = g0[1:8] + g0[10:]
            pending = []
            for h in range(H):
                if h % 2 == 0 and h < H - 2:
                    pending = proj_group(h // 2 + 1)
                fillers = list(pre)
                pre = []
                if h % 2 == 0:
                    take = (len(pending) + 1) // 2
                else:
                    take = len(pending)
                fillers += pending[:take]
                pending = pending[take:]
                attention(h, fillers, rate=2 if h == 0 else 1)
                pre = [lambda st=2 * h: outproj(st),
                       lambda st=2 * h + 1: outproj(st)]
                if h == 5:
                    pre.append(lambda: finalize(0, 12))
            for w in pre:
                w()
            finalize(12, NT)

    return nc


def _get_nc():
    if "nc" not in _CACHE:
        nc = _build()
        _legalize_sync_waits(nc)
        _CACHE["nc"] = nc
    return _CACHE["nc"]


def _prep_in_maps(x, Wq, bq, Wk, bk, Wv, bv, Wo, bo, gamma, beta):
    sc = 1.0 / np.sqrt(DIN)
    f32 = np.float32
    wq16 = (np.asarray(Wq, f32) * sc).astype(np.float16)
    wk16 = np.asarray(Wk, f32).astype(np.float16)
    wv16 = np.asarray(Wv, f32).astype(np.float16)
    # wo[d, c, dout] = Wo[c*64+d, dout]
    wo3 = np.asarray(Wo, f32).astype(np.float16).reshape(H, DIN, DIN) \
        .transpose(1, 0, 2).copy()
    bq2 = (np.asarray(bq, f32) * sc).reshape(H, DIN).T.copy()
    bk2 = np.asarray(bk, f32).reshape(H, DIN).T.copy()
    bv2 = np.asarray(bv, f32).astype(np.float16).reshape(1, HD)
    bo2 = np.asarray(bo, f32).astype(np.float16).reshape(1, DIN)
    gb = np.ascontiguousarray(np.broadcast_to(np.asarray(gamma, f32), (P, DIN)))
    bb = np.ascontiguousarray(np.broadcast_to(np.asarray(beta, f32), (P, DIN)))

    in_maps = []
    B = x.shape[0]
    for b in range(B):
        xb = np.asarray(x[b], f32)
        x3 = np.ascontiguousarray(xb.reshape(NT, P, DIN).transpose(1, 0, 2))
        xt16 = np.ascontiguousarray(xb.T).astype(np.float16)
        in_maps.append(dict(
            x=x3, xt=xt16, wq=wq16, wk=wk16, wv=wv16, wo=wo3,
            bq=bq2, bk=bk2, bv=bv2, bo=bo2, gamma=gb, beta=bb,
        ))
    return in_maps


def run(trace=False, **inputs):
    nc = _get_nc()
    in_maps = _prep_in_maps(**inputs)
    res = run_bass_kernel_spmd(
        nc, in_maps, core_ids=list(range(len(in_maps))), trace=trace,
    )
    B = len(in_maps)
    y = np.stack([res.results[b]["y"] for b in range(B)])[:, None]
    return np.asarray(y, np.float32), res


def kernel(**inputs):
    y, _ = run(trace=False, **inputs)
    return y


# revision 36
# speedup vs baseline: 2197.5786x; 2197.5786x over previous
"""Trainium2 Bass kernel for nn_MultiHeadAttention_T_4818953306886.

Reference semantics (B=8, S=2048, D=64, H=8, HD=512):
    q = (x @ Wq + bq).reshape(B*H, S, D)      # raw row-major view!
    k, v likewise
    attn = softmax(q @ k^T / sqrt(D), axis=2)
    ctx = attn @ v                             # [B*H, S, D]
    out = ctx.reshape(B, 1, S, HD) @ Wo + bo   # [B, 1, S, D]
    y = LayerNorm(x[:, None] + out) * gamma + beta

The raw reshape means head h attends over a permuted view of rows
h*256..h*256+255 of the projection output: with j' = c*256 + r,
    Q'_h[c*256+r, d] = Q[h*256+r, c*64+d]
and softmax attention is invariant to the (shared) permutation of the
key/value index, so we work entirely in (c, r) block layout.

Sharding: data-parallel over batch, one batch per core, weights replicated.

Dataflow per core (matmul operands fp16, accumulation fp32):
  xT   (64, 2048)        x^T (host-pretransposed input)
  QTd  (64, c=8, 2048)   Q^T per 64-wide column chunk (1/sqrt(D) folded in)
  KTd  same for K
  Vaug (128, h=8, jc=16, 65)  V' j'-chunks with a ones column appended
  per (head, i'-tile of 512):
    scoresT (j'=128, i'=512) via K-stationary matmuls -> PSUM
    E = exp(scoresT) on ScalarE (PSUM -> SBUF fp16)
    ctxT_aug (65, 512) accumulated PV matmul; row 64 = softmax denominator
    normalize: DVE reciprocal + partition-broadcast + one DVE multiply
  out-projection per s-tile of 128 (accumulated over the 8 c-chunks) + bo
  (rank-1 matmul), + residual; LayerNorm with a single batched
  rstd = exp(-0.5*ln(var+eps)) pass over all 16 s-tiles.

Projection groups (one 512-wide s-tile of Q/K + 4 V s-tiles) are emitted
interleaved with the head loop so the ScalarE exp stream starts early and
never starves; PSUM stays within the 8-bank budget:
  scores 2x(2 banks) + ctx 2x(1) + aux(proj/outproj/bcast) 2x(1) = 8.
"""

import numpy as np

import concourse.bass as bass
import concourse.tile as tile
from concourse import mybir
from concourse.bass_utils import run_bass_kernel_spmd

F32 = mybir.dt.float32
F16 = mybir.dt.float16

S = 2048          # sequence length per batch
DIN = 64          # model dim
H = 8             # heads
HD = 512          # H * DIN
NT = 16           # s-tiles of 128
P = 128
LN_EPS = 1e-5
AF = mybir.ActivationFunctionType

# scores jc-chunks per exp op: 2 -> (128, 1024) fp32 = 2 PSUM banks
SJC = 2
# use GPSIMD stride-0 partition broadcast for the softmax denominator;
# if False, use a rank-1 PE matmul + DVE copy instead.
GPSIMD_BCAST = False

_CACHE = {}

# walrus in this container accepts at most 1 sync-wait command per engine
# instruction and at most 2 per EventSemaphore. Tile packs every dependency
# onto the consuming instruction, so hoist the excess onto standalone
# EventSemaphore instructions inserted just before it (same engine stream).
_MAX_EV_WAITS = 2


def _legalize_sync_waits(nc, keep=1):
    n_fixed = 0
    for function in nc.m.functions:
        for block in function.blocks:
            out = []
            changed = False
            for inst in block.instructions:
                si = getattr(inst, "sync_info", None)
                waits = list(si.on_wait) if (si is not None and si.on_wait) else []
                if len(waits) > keep and not isinstance(
                        inst, mybir.InstEventSemaphore):
                    extra = waits[:-keep] if keep else waits
                    kept = waits[-keep:] if keep else []
                    for ci in range(0, len(extra), _MAX_EV_WAITS):
                        ev = mybir.InstEventSemaphore(
                            name=f"{inst.name}-w{ci}", ins=[], outs=[],
                            sync_info=mybir.SyncInfo(
                                on_wait=extra[ci:ci + _MAX_EV_WAITS],
                                on_update=[]),
                        )
                        ev.engine = inst.engine
                        out.append(ev)
                    inst.sync_info = mybir.SyncInfo(
                        on_wait=kept, on_update=list(si.on_update))
                    changed = True
                    n_fixed += 1
                out.append(inst)
            if changed:
                block.instructions = out
    return n_fixed


def _build():
    nc = bass.Bass()

    x_d = nc.dram_tensor("x", [P, NT, DIN], F32, kind="ExternalInput")
    xt_d = nc.dram_tensor("xt", [DIN, S], F16, kind="ExternalInput")
    wq_d = nc.dram_tensor("wq", [DIN, HD], F16, kind="ExternalInput")
    wk_d = nc.dram_tensor("wk", [DIN, HD], F16, kind="ExternalInput")
    wv_d = nc.dram_tensor("wv", [DIN, HD], F16, kind="ExternalInput")
    wo_d = nc.dram_tensor("wo", [DIN, H, DIN], F16, kind="ExternalInput")
    bq_d = nc.dram_tensor("bq", [DIN, H], F32, kind="ExternalInput")
    bk_d = nc.dram_tensor("bk", [DIN, H], F32, kind="ExternalInput")
    bv_d = nc.dram_tensor("bv", [1, HD], F16, kind="ExternalInput")
    bo_d = nc.dram_tensor("bo", [1, DIN], F16, kind="ExternalInput")
    gam_d = nc.dram_tensor("gamma", [P, DIN], F32, kind="ExternalInput")
    bet_d = nc.dram_tensor("beta", [P, DIN], F32, kind="ExternalInput")
    y_d = nc.dram_tensor("y", [S, DIN], F32, kind="ExternalOutput")

    with tile.TileContext(nc) as tc:
        with (
            tc.tile_pool(name="consts", bufs=1) as consts,
            tc.tile_pool(name="spool", bufs=2, space=bass.MemorySpace.PSUM) as spool,
            tc.tile_pool(name="cpool", bufs=2, space=bass.MemorySpace.PSUM) as cpool,
            tc.tile_pool(name="aux", bufs=2, space=bass.MemorySpace.PSUM) as aux,
            tc.tile_pool(name="epool", bufs=2) as epool,
            tc.tile_pool(name="wpool", bufs=2) as wpool,
            tc.tile_pool(name="lpool", bufs=3) as lpool,
        ):
            ones0 = consts.tile([1, P], F16)
            nc.vector.memset(ones0[:], 1.0)
            ones2 = consts.tile([65, P], F16)
            nc.vector.memset(ones2[64:65, :], 1.0)
            eps_t = consts.tile([P, 1], F32)
            nc.vector.memset(eps_t, LN_EPS)
            # dummy exp: trigger the ACT table load during the prologue
            warm = consts.tile([P, 1], F32)
            nc.scalar.activation(warm[:], eps_t[:], AF.Exp)

            # DMA order matters: xT + K/Q weights + their biases gate the
            # critical path (first projections -> first scores -> first exp).
            xT = consts.tile([DIN, S], F16)
            nc.sync.dma_start(xT[:], xt_d[:])
            wk_sb = consts.tile([DIN, HD], F16)
            nc.sync.dma_start(wk_sb[:], wk_d[:])
            wq_sb = consts.tile([DIN, HD], F16)
            nc.sync.dma_start(wq_sb[:], wq_d[:])
            bq_sb = consts.tile([DIN, H], F32)
            nc.sync.dma_start(bq_sb[:], bq_d[:])
            bk_sb = consts.tile([DIN, H], F32)
            nc.sync.dma_start(bk_sb[:], bk_d[:])
            wv_sb = consts.tile([DIN, HD], F16)
            nc.sync.dma_start(wv_sb[:], wv_d[:])
            bv_sb = consts.tile([1, HD], F16)
            nc.sync.dma_start(bv_sb[:], bv_d[:])
            wo_sb = consts.tile([DIN, H, DIN], F16)
            nc.sync.dma_start(wo_sb[:], wo_d[:])
            bo_sb = consts.tile([1, DIN], F16)
            nc.sync.dma_start(bo_sb[:], bo_d[:])
            gamma_b = consts.tile([P, DIN], F32)
            nc.sync.dma_start(gamma_b[:], gam_d[:])
            beta_b = consts.tile([P, DIN], F32)
            nc.sync.dma_start(beta_b[:], bet_d[:])
            x_res = consts.tile([P, NT, DIN], F32)
            nc.sync.dma_start(x_res[:], x_d[:])

            QTd = consts.tile([DIN, H, S], F16)
            KTd = consts.tile([DIN, H, S], F16)
            Vaug = consts.tile([P, H, NT, 65], F16)
            nc.vector.memset(Vaug[:, :, :, 64:65], 1.0)
            ctxT = consts.tile([DIN, H, S], F16)

            y_all = consts.tile([P, NT, DIN], F32)
            mv_all = consts.tile([P, NT, 2], F32)
            rstd_all = consts.tile([P, NT], F32)

            def proj_group(g):
                """Q^T, K^T chunks for s-columns [g*512,(g+1)*512) and
                V'/Vaug rows for heads 2g, 2g+1 — as a list of small
                closures (~1 matmul + evacuation each) so they can be
                sprinkled between score-groups without starving ScalarE."""
                sl = slice(g * 512, (g + 1) * 512)
                work = []

                def qk(w_sb, b_sb, out_t, c):
                    # NOTE: evacuating via ScalarE activation(Identity, bias=AP)
                    # produced NaNs on hardware — keep this on the DVE.
                    ps = aux.tile([DIN, 512], F32, tag="aux")
                    nc.tensor.matmul(
                        ps[:], w_sb[:, c * DIN:(c + 1) * DIN], xT[:, sl],
                        start=True, stop=True)
                    nc.vector.tensor_scalar_add(
                        out_t[:, c, sl], ps[:], b_sb[:, c:c + 1])

                def vproj(k):
                    st = 4 * g + k
                    h, half = st // 2, st % 2
                    ps = aux.tile([P, HD], F32, tag="aux")
                    nc.tensor.matmul(
                        ps[:], xT[:, st * P:(st + 1) * P], wv_sb[:],
                        start=True, stop=False)
                    nc.tensor.matmul(
                        ps[:], ones0[:], bv_sb[:], start=False, stop=True)
                    # psum[p, c*64+d] -> Vaug[p, h, 2c+half, d]
                    dst = bass.AP(
                        tensor=Vaug.tensor,
                        offset=Vaug.offset + (h * NT + half) * 65,
                        ap=[[Vaug.ap[0][0], P], [2 * 65, H], [1, DIN]],
                    )
                    nc.vector.tensor_copy(
                        dst, ps[:].rearrange("p (c d) -> p c d", d=DIN))

                # K chunks first (score-group g of every i'-tile needs K
                # chunk c=g), then V (needed by the first PV accumulation),
                # then remaining Q chunks (needed pairwise per i'-tile).
                for c in range(H):
                    work.append(lambda c=c: qk(wk_sb, bk_sb, KTd, c))
                for c in range(2):
                    work.append(lambda c=c: qk(wq_sb, bq_sb, QTd, c))
                for k in range(4):
                    work.append(lambda k=k: vproj(k))
                for c in range(2, H):
                    work.append(lambda c=c: qk(wq_sb, bq_sb, QTd, c))
                return work

            def attention(h, fillers, rate=1):
                fi = 0
                for it in range(4):
                    rhs_q = QTd[:, 2 * it:2 * it + 2, h * 256:(h + 1) * 256]
                    E_t = epool.tile([P, NT, 512], F16, tag="E")
                    for g in range(NT // SJC):
                        ps = spool.tile([P, SJC * 512], F32, tag="sc")
                        for jg in range(SJC):
                            jc = g * SJC + jg
                            off = h * 256 + (jc % 2) * P
                            nc.tensor.matmul(
                                ps[:, jg * 512:(jg + 1) * 512],
                                KTd[:, jc // 2, off:off + P],
                                rhs_q, start=True, stop=True)
                        nc.scalar.activation(
                            E_t[:, g * SJC:(g + 1) * SJC, :], ps[:], AF.Exp)
                        for _ in range(rate):
                            if fi < len(fillers):
                                fillers[fi]()
                                fi += 1
                    pc = cpool.tile([65, 512], F32, tag="ctx")
                    for jc in range(NT):
                        nc.tensor.matmul(
                            pc[:], Vaug[:, h, jc, :], E_t[:, jc, :],
                            start=(jc == 0), stop=(jc == NT - 1))
                    # row 64 of pc is the softmax denominator over j'
                    rs = wpool.tile([65, 512], F32, tag="r32")
                    nc.vector.reciprocal(rs[64:65, :], pc[64:65, :])
                    if GPSIMD_BCAST:
                        bc = wpool.tile([DIN, 512], F32, tag="bcs")
                        row = rs[64:65, :]
                        src = bass.AP(tensor=row.tensor, offset=row.offset,
                                      ap=[[0, DIN]] + [list(a) for a in row.ap[1:]])
                        nc.gpsimd.tensor_copy(bc[:], src)
                    else:
                        rs16 = wpool.tile([65, 512], F16, tag="r16")
                        nc.vector.tensor_copy(rs16[64:65, :], rs[64:65, :])
                        pb = aux.tile([DIN, 512], F32, tag="aux")
                        nc.tensor.matmul(
                            pb[:], ones2[64:65, 0:DIN], rs16[64:65, :],
                            start=True, stop=True)
                        bc = wpool.tile([DIN, 512], F16, tag="bcs")
                        nc.vector.tensor_copy(bc[:], pb[:])
                    nc.vector.tensor_mul(
                        ctxT[:, 2 * it:2 * it + 2, h * 256:(h + 1) * 256],
                        pc[0:DIN, :].rearrange("d (c r) -> d c r", r=256),
                        bc[:].rearrange("d (c r) -> d c r", r=256))


            def outproj(st):
                po = aux.tile([P, DIN], F32, tag="aux")
                for c in range(H):
                    nc.tensor.matmul(
                        po[:], ctxT[:, c, st * P:(st + 1) * P],
                        wo_sb[:, c, :], start=(c == 0), stop=False)
                nc.tensor.matmul(
                    po[:], ones0[:], bo_sb[:], start=False, stop=True)
                nc.vector.tensor_add(y_all[:, st, :], po[:], x_res[:, st, :])
                stats = lpool.tile([P, 6], F32, tag="st")
                nc.vector.bn_stats(stats[:], y_all[:, st, :])
                nc.vector.bn_aggr(mv_all[:, st, :], stats[:])

            lnv = consts.tile([P, NT], F32)

            def finalize(st0, st1):
                """LayerNorm rstd = exp(-0.5*ln(var+eps)) for s-tiles
                [st0, st1), then normalize + affine + store."""
                nc.scalar.activation(
                    lnv[:, st0:st1], mv_all[:, st0:st1, 1], AF.Ln,
                    bias=eps_t[:])
                nc.scalar.activation(
                    rstd_all[:, st0:st1], lnv[:, st0:st1], AF.Exp, scale=-0.5)
                for st in range(st0, st1):
                    yn = lpool.tile([P, DIN], F32, tag="yn")
                    nc.vector.tensor_scalar(
                        yn[:], y_all[:, st, :],
                        scalar1=mv_all[:, st, 0:1],
                        scalar2=rstd_all[:, st:st + 1],
                        op0=mybir.AluOpType.subtract,
                        op1=mybir.AluOpType.mult)
                    nc.vector.tensor_mul(yn[:], yn[:], gamma_b[:])
                    yo = lpool.tile([P, DIN], F32, tag="yo")
                    nc.vector.tensor_add(yo[:], yn[:], beta_b[:])
                    nc.sync.dma_start(y_d[st * P:(st + 1) * P, :], yo[:])

            # projection group 0: K c0 + Q c0,c1 run up front (the minimum
            # head 0's first score-group needs); everything else — remaining
            # projections, the next group, the previous head's output
            # projections, and the first LayerNorm finalize — is sprinkled
            # through the head loop as fillers so the PE never takes a
            # detour longer than ScalarE's score backlog.
            g0 = proj_group(0)
            g0[0]()   # K c0
            g0[8]()   # Q c0
            g0[9]()   # Q c1
            pre = g0[1:8] + g0[10:]
            pending = []
            for h in range(H):
                if h % 2 == 0 and h < H - 2:
                    pending = proj_group(h // 2 + 1)
                fillers = list(pre)
                pre = []
                if h % 2 == 0:
                    take = (len(pending) + 1) // 2
                else:
                    take = len(pending)
                fillers += pending[:take]
                pending = pending[take:]
                attention(h, fillers, rate=2 if h == 0 else 1)
                pre = [lambda st=2 * h: outproj(st),
                       lambda st=2 * h + 1: outproj(st)]
                if h == 5:
                    pre.append(lambda: finalize(0, 12))
            for w in pre:
                w()
            finalize(12, NT)

    return nc


def _get_nc():
    if "nc" not in _CACHE:
        nc = _build()
        _legalize_sync_waits(nc)
        _CACHE["nc"] = nc
    return _CACHE["nc"]


def _prep_in_maps(x, Wq, bq, Wk, bk, Wv, bv, Wo, bo, gamma, beta):
    sc = 1.0 / np.sqrt(DIN)
    f32 = np.float32
    wq16 = (np.asarray(Wq, f32) * sc).astype(np.float16)
    wk16 = np.asarray(Wk, f32).astype(np.float16)
    wv16 = np.asarray(Wv, f32).astype(np.float16)
    # wo[d, c, dout] = Wo[c*64+d, dout]
    wo3 = np.asarray(Wo, f32).astype(np.float16).reshape(H, DIN, DIN) \
        .transpose(1, 0, 2).copy()
    bq2 = (np.asarray(bq, f32) * sc).reshape(H, DIN).T.copy()
    bk2 = np.asarray(bk, f32).reshape(H, DIN).T.copy()
    bv2 = np.asarray(bv, f32).astype(np.float16).reshape(1, HD)
    bo2 = np.asarray(bo, f32).astype(np.float16).reshape(1, DIN)
    gb = np.ascontiguousarray(np.broadcast_to(np.asarray(gamma, f32), (P, DIN)))
    bb = np.ascontiguousarray(np.broadcast_to(np.asarray(beta, f32), (P, DIN)))

    in_maps = []
    B = x.shape[0]
    for b in range(B):
        xb = np.asarray(x[b], f32)
        x3 = np.ascontiguousarray(xb.reshape(NT, P, DIN).transpose(1, 0, 2))
        xt16 = np.ascontiguousarray(xb.T).astype(np.float16)
        in_maps.append(dict(
            x=x3, xt=xt16, wq=wq16, wk=wk16, wv=wv16, wo=wo3,
            bq=bq2, bk=bk2, bv=bv2, bo=bo2, gamma=gb, beta=bb,
        ))
    return in_maps


def run(trace=False, **inputs):
    nc = _get_nc()
    in_maps = _prep_in_maps(**inputs)
    res = run_bass_kernel_spmd(
        nc, in_maps, core_ids=list(range(len(in_maps))), trace=trace,
    )
    B = len(in_maps)
    y = np.stack([res.results[b]["y"] for b in range(B)])[:, None]
    return np.asarray(y, np.float32), res


def kernel(**inputs):
    y, _ = run(trace=False, **inputs)
    return y
